# revision 16
# baseline (speedup 1.0000x reference)
"""DisagreementRegularizer Trainium2 kernel.

reference math:
    xn = x / max(||x||_2 along d, eps)
    sim[b,q,p] = xn[b,q,:] . xn[b,p,:]
    out[b] = -mean_{q,p} sim  =  -(1/Q^2) * || sum_q xn[b,q,:] ||^2

Per batch b (on device):
    sumsq[q] = sum_d x[q,d]^2            (fused square+row-reduce, see below)
    rnorm[q] = sqrt(1/sumsq[q])          (DVE reciprocal + ACT Sqrt)
    s[d]     = sum_q rnorm[q]*x[q,d]     (PE matmul, rnorm as stationary weights)
Host: out[b] = -(1/Q^2) * sum_d s[b,d]^2   (tiny per-core finish)

v4 (45.6us baseline / 47.1 v2 / 52.6 v3-pool). Measured facts:
  * load stream = HBM line rate (~381 GB/s, ~21us / 8 MiB); a load's data
    is usable ~0.9us after its last byte (DMA sem prop) -> per-batch loads.
  * HWDGE (sync) loads start ~2us before the SWDGE path warms up, but
    cannot cast -> batches 0 and 1 load fp32 via HWDGE and run their whole
    pipeline fp32 (PE does 2-pass fp32 matmuls for them; PE has slack).
    Batches 2..15 cast fp32->bf16 in SWDGE loads.
  * fused square+row-reduce per (batch,chunk) [128,256] segment: DVE
    scalar_tensor_tensor 423+44ns, ACT Square+accum_out 507+186ns.
    (tensor_tensor_reduce wedges the device - never emit it.)
    64 segments split ~33 DVE / ~27 ACT to equalize engine busy (~21us
    each vs the ~21us stream); tail batches split 2+2 so their chains
    run on both engines in parallel.
  * the gpsimd Q7 is NOT free mid-stream (SWDGE descriptor emission is
    ring-paced until ~4us before stream end) -> only batch 11, arriving
    late, is squared on the Pool engine (tensor_mul, ~1.9us) with its
    segmented reduce on DVE.
  * s cells: batch bb -> PSUM (row 32*(bb%3), col block bb//3); matmul
    out APs may only be based at partitions 0/32/64.  Cell col blocks are
    copied PSUM->SBUF as their last batch finishes (free-dim cost only)
    and shipped in 4 DMAs, so the final copy+store is tiny.  PSUM is
    memset once during the idle preamble (never-written partitions are
    read by the copies).  Host unscrambles the cells.
  * rnorm chains are emitted 1-2 units late so cross-engine waits never
    head-of-line-block the next unit's ops on an in-order engine.

Sharding: pure data parallel, batch dim 128 -> 16 per core across 8 cores.
"""

import numpy as np

B, Q, D = 128, 512, 256
N_CORES = 8
BL = B // N_CORES  # 16 batches per core
CHUNKS = 4  # Q = 512 = 128 partitions x 4 chunks
EPS = 1e-12

F32_BATCHES = {0, 1}              # HWDGE fp32 loads + fp32 pipeline
POOL_BATCHES = set()              # (Q7 is ring-paced busy; pool hurt)
ACT1_BATCHES = {6, 8, 10, 12, 13, 14}  # only c3 on ACT (tail stays ACT-light)
ACT3_BATCHES = {3, 5, 7}          # c1,c2,c3 on ACT (mid-stream, ACT absorbs)
# everything else: c0,c2 on DVE and c1,c3 on ACT (b0/b15 via halves)


def _cell(bb):
    if bb >= 13:
        return 32 * (bb - 13), 5 * D
    return 32 * (bb % 3), (bb // 3) * D


def _build(nc):
    import concourse.mybir as mybir
    import concourse.tile as tile

    f32 = mybir.dt.float32
    f16 = mybir.dt.bfloat16
    Act = mybir.ActivationFunctionType
    Alu = mybir.AluOpType

    x_d = nc.dram_tensor("x", [BL, Q, D], f32, kind="ExternalInput").ap()
    s_d = nc.dram_tensor("s_out", [3, 6 * D], f32, kind="ExternalOutput").ap()

    with tile.TileContext(nc) as tc:
        with (
            tc.tile_pool(name="xp", bufs=1) as xp,
            tc.tile_pool(name="scr", bufs=1) as scrp,
            tc.tile_pool(name="sqp", bufs=2) as sqp,
            tc.tile_pool(name="small", bufs=1) as small,
            tc.tile_pool(name="fin", bufs=1) as fin,
            tc.tile_pool(name="ps", bufs=1, space="PSUM") as psp,
        ):
            s_ps = psp.tile([96, 6 * D], f32)

            # ---- loads (issued upfront; HWDGE first for the early start) --
            x_tiles = {}
            src0 = x_d[0:1].rearrange("b (p c) d -> p b c d", p=128)
            x0a = xp.tile([128, 1, 2, D], f32, tag="x0a")
            x0b = xp.tile([128, 1, 2, D], f32, tag="x0b")
            nc.sync.dma_start(out=x0a[:], in_=src0[:, :, 0:2])
            nc.sync.dma_start(out=x0b[:], in_=src0[:, :, 2:4])
            x_tiles[0] = (x0a, x0b)
            src1 = x_d[1:2].rearrange("b (p c) d -> p b c d", p=128)
            x1 = xp.tile([128, 1, CHUNKS, D], f32, tag="x1")
            nc.sync.dma_start(out=x1[:], in_=src1)
            x_tiles[1] = x1
            for bb in range(2, BL):
                src = x_d[bb : bb + 1].rearrange("b (p c) d -> p b c d", p=128)
                if bb == BL - 1:
                    a = xp.tile([128, 1, 2, D], f16, tag="x15a")
                    b = xp.tile([128, 1, 2, D], f16, tag="x15b")
                    nc.gpsimd.dma_start(out=a[:], in_=src[:, :, 0:2])
                    nc.gpsimd.dma_start(out=b[:], in_=src[:, :, 2:4])
                    x_tiles[bb] = (a, b)
                else:
                    t = xp.tile([128, 1, CHUNKS, D], f16, tag=f"x{bb}")
                    nc.gpsimd.dma_start(out=t[:], in_=src)
                    x_tiles[bb] = t

            # zero PSUM during the idle preamble/load window (epilogue
            # copies read never-written partitions; they must see zeros)
            nc.vector.memset(s_ps[:], 0.0)

            # dummy Sqrt pins the ACT table set (sqrt_and_others has
            # Square+Sqrt+Copy) -> exactly one ACT_TABLE_LOAD
            dummy = small.tile([1, 1], f32, tag="dummy")
            nc.vector.memset(dummy[:], 1.0)
            dummy2 = small.tile([1, 1], f32, tag="dummy2")
            nc.scalar.activation(out=dummy2[:], in_=dummy[:], func=Act.Sqrt)

            scr_d = scrp.tile([128, D], f16, tag="scr_d")
            scr_a = scrp.tile([128, D], f16, tag="scr_a")
            scr_d32 = scrp.tile([128, D], f32, tag="scr_d32")
            scr_a32 = scrp.tile([128, D], f32, tag="scr_a32")

            def x_seg(bb, c):
                t = x_tiles[bb]
                if isinstance(t, tuple):
                    return t[c // 2][:, 0, c % 2, :]
                return t[:, 0, c, :]

            def x_flat(bb):
                return x_tiles[bb][:].rearrange("p b c d -> p (b c d)")

            # ---- units: one per batch; b0/b15 have two half-units ----
            units = [(0, 0), (0, 1)]
            units += [(bb, None) for bb in range(1, BL - 1)]
            units += [(BL - 1, 0), (BL - 1, 1)]
            u_of_batch = {0: 1, BL - 1: len(units) - 1}
            for bb in range(1, BL - 1):
                u_of_batch[bb] = bb + 1

            # rnorm blocks (unit lists); b0/b1 fp32 blocks separate
            blocks = [
                [0], [1], [u_of_batch[1]],
                [u_of_batch[2], u_of_batch[3], u_of_batch[4]],
                [u_of_batch[5], u_of_batch[6], u_of_batch[7]],
                [u_of_batch[8], u_of_batch[9], u_of_batch[10]],
                [u_of_batch[11]], [u_of_batch[12]],
                [u_of_batch[13]], [u_of_batch[14]],
                [len(units) - 2], [len(units) - 1],
            ]
            unit_of = {}
            for bi, us in enumerate(blocks):
                for u in us:
                    unit_of[u] = bi

            blk_info = {}
            for bi, us in enumerate(blocks):
                segs = []
                for u in us:
                    bb, h = units[u]
                    cs = range(2 * h, 2 * h + 2) if h is not None else range(4)
                    for c in cs:
                        segs.append((bb, c, len(segs)))
                is32 = units[us[0]][0] in F32_BATCHES
                blk_info[bi] = dict(segs=segs, is32=is32, sumsq=None)

            def use_act(bb, c, h):
                if h is not None:
                    return c % 2 == 1
                if bb in ACT1_BATCHES:
                    return c == 3
                if bb in ACT3_BATCHES:
                    return c != 0
                return c % 2 == 1

            def emit_fused(u):
                bb, h = units[u]
                bi = unit_of[u]
                info = blk_info[bi]
                if info["sumsq"] is None:
                    info["sumsq"] = small.tile(
                        [128, len(info["segs"])], f32,
                        tag=f"sumsq{bi}", name=f"sumsq{bi}",
                    )
                sumsq = info["sumsq"]
                col0 = next(col for (b2, c2, col) in info["segs"] if b2 == bb)

                if bb in POOL_BATCHES:
                    sq = sqp.tile([128, CHUNKS * D], f16, tag="sq_pool")
                    nc.gpsimd.tensor_mul(sq[:], x_flat(bb), x_flat(bb))
                    return (sq, sumsq, col0)

                is32 = bb in F32_BATCHES
                cs = range(2 * h, 2 * h + 2) if h is not None else range(4)
                for c in cs:
                    seg = x_seg(bb, c)
                    col = col0 + (c - (2 * h if h is not None else 0))
                    acc = sumsq[:, col : col + 1]
                    if use_act(bb, c, h):
                        nc.scalar.activation(
                            out=(scr_a32 if is32 else scr_a)[:],
                            in_=seg, func=Act.Square, accum_out=acc,
                        )
                    else:
                        nc.vector.scalar_tensor_tensor(
                            out=(scr_d32 if is32 else scr_d)[:],
                            in0=seg, scalar=1.0, in1=seg,
                            op0=Alu.mult, op1=Alu.mult, accum_out=acc,
                        )
                return None

            def emit_deferred_reduce(dr):
                sq, sumsq, col0 = dr
                nc.vector.tensor_reduce(
                    out=sumsq[:, col0 : col0 + CHUNKS],
                    in_=sq[:].rearrange("p (s d) -> p s d", d=D),
                    axis=mybir.AxisListType.X,
                    op=Alu.add,
                )

            def emit_rnorm_and_mm(bi):
                info = blk_info[bi]
                n = len(info["segs"])
                wdt = f32 if info["is32"] else f16
                with tc.high_priority():
                    rsum = small.tile([128, n], f32, tag=f"rsum{bi}",
                                      name=f"rsum{bi}")
                    nc.vector.reciprocal(out=rsum[:], in_=info["sumsq"][:])
                    rnorm = small.tile([128, n], wdt, tag=f"rnorm{bi}",
                                       name=f"rnorm{bi}")
                    nc.scalar.activation(out=rnorm[:], in_=rsum[:], func=Act.Sqrt)
                for bb, c, col in info["segs"]:
                    r, j = _cell(bb)
                    nc.tensor.matmul(
                        s_ps[r : r + 1, j : j + D],
                        rnorm[:, col : col + 1],
                        x_seg(bb, c),
                        start=(c == 0),
                        stop=(c == CHUNKS - 1),
                    )

            # ---- progressive epilogue ----
            s_sb = fin.tile([96, 6 * D], f32, tag="s_sb")
            rows = s_sb[:].rearrange("(r k) f -> r k f", r=3)[:, 0, :]

            def emit_copy(j, eng):
                c0, c1 = j * D, (j + 1) * D
                if eng == "dve":
                    nc.vector.tensor_copy(s_sb[:, c0:c1], s_ps[:, c0:c1])
                else:
                    nc.scalar.copy(s_sb[:, c0:c1], s_ps[:, c0:c1])

            # j-block -> (gating rnorm block, engine); j5 at the end
            blk_of_batch = {}
            for bi, us in enumerate(blocks):
                for u in us:
                    blk_of_batch[units[u][0]] = bi
            copy_after_block = {}
            for j, (gate_bb, eng) in enumerate(
                [(2, "dve"), (5, "act"), (8, "dve"), (11, "act"), (12, "dve")]
            ):
                copy_after_block.setdefault(blk_of_batch[gate_bb], []).append(
                    (j, eng)
                )
            dma_after_j = {1: (0, 512), 3: (512, 1024), 4: (1024, 1280)}

            # ---- emission schedule ----
            pending_rnorm = {}
            for bi, us in enumerate(blocks):
                has_late = any(units[u][0] in POOL_BATCHES for u in us)
                pending_rnorm[bi] = us[-1] + (2 if has_late else 1)
            deferred = []
            emitted = set()

            def flush(upto):
                nonlocal deferred
                rest = []
                for at, dr in deferred:
                    if at <= upto:
                        emit_deferred_reduce(dr)
                    else:
                        rest.append((at, dr))
                deferred = rest
                for bi in range(len(blocks)):
                    if bi in emitted or pending_rnorm[bi] > upto:
                        continue
                    emitted.add(bi)
                    emit_rnorm_and_mm(bi)
                    for j, eng in copy_after_block.get(bi, []):
                        emit_copy(j, eng)
                        if j in dma_after_j:
                            a, b = dma_after_j[j]
                            nc.sync.dma_start(
                                out=s_d[0:3, a:b], in_=rows[:, a:b]
                            )

            for u in range(len(units)):
                dr = emit_fused(u)
                if dr is not None:
                    deferred.append((u + 1, dr))
                flush(u)
            flush(10**9)

            emit_copy(5, "dve")
            nc.sync.dma_start(out=s_d[0:3, 1280:1536], in_=rows[:, 1280:1536])
    return nc


def _make_nc():
    import concourse.bacc as bacc

    nc = bacc.Bacc(trn_type="TRN2")
    _build(nc)
    nc.finalize()
    return nc


def _finish(s):
    # s: [3, 6*D] cell grid; batch bb at _cell(bb). out[b] = -(1/Q^2)*||s_b||^2
    s = s.astype(np.float32).reshape(3, 6, D)
    v = np.empty((BL, D), np.float32)
    for bb in range(BL):
        r, j = _cell(bb)
        v[bb] = s[r // 32, j // D]
    return -(v * v).sum(axis=-1) / np.float32(Q * Q)


def _run(x, trace=False):
    from concourse.bass_utils import run_bass_kernel_spmd

    in_maps = [
        {"x": np.ascontiguousarray(x[i * BL : (i + 1) * BL])} for i in range(N_CORES)
    ]
    nc = _make_nc()
    res = run_bass_kernel_spmd(
        nc, in_maps, core_ids=list(range(N_CORES)), trace=trace
    )
    out = np.concatenate([_finish(r["s_out"]) for r in res.results], axis=0)
    return out.astype(np.float32), res


def kernel(x: np.ndarray) -> np.ndarray:
    out, _ = _run(np.asarray(x, dtype=np.float32))
    return out
